# revision 15
# baseline (speedup 1.0000x reference)
"""Trainium2 Bass kernel: per-combination linear encoder via PE one-hot
matmuls (no GPSIMD gather).

z = y * w[idx] + b[idx],  idx = t*1024 + x @ [512..1]  (11 bits, 2048 combos)

Split idx = hi5*64 + lo6 (hi5 = t,x0..x3; lo6 = x4..x9). Per tile of
1024 rows (2 groups g of 512 columns, rows on the FREE axis):

  S1a  PE   u_lo[64g+l, c]   = lo6(r) - l       (block-diag affine MM ->
  S1b  PE   u_hi[64g+2h+e,c] = hi5(r) - h        one [128,2*NT] psum pair,
                                                 e-duplicated for w/b lanes)
  cp   ACT  uu -> bf16 SBUF (single merged psum drain, exact: |u| < 64)
  S2   DVE  oh = (u_lo == 0)          bf16, 4x mode
  S3   PE   V[64g+2h+e, c] = sum_l T[l,2h+e]*oh    (T = w/b tables, bf16)
  S4   DVE  msk = (u_hi == 0) * V     (fused scalar_tensor_tensor)
  sel  PE   8 tiles accumulate into sel8[48,512]: tile u writes w to
            slot 2u+g, b to slot 32+2u+g via slot-shifted one-hot
            stationaries (M=48, N=512, one accumulation group per pack)
  FMA  DVE  z[16,512] = y16 * sel8[0:16] + sel8[32:48]; DMA out.
Matmuls are pair-batched (same stationary back to back) to cut
LDWEIGHTS switches; ~516 us on 8 cores (PE-paced).

All row/column packing is host-side layout only; the device does all
arithmetic. w/b are bf16 (0.2% rounding, tolerance is 2e-2).
"""

import numpy as np
import ml_dtypes

import concourse.bacc as bacc
import concourse.mybir as mybir
from concourse.tile import TileContext
from concourse.bass_utils import run_bass_kernel_spmd

M = 8              # NeuronCores
NT = 512           # columns per tile (one PSUM bank)
G = 2              # row-groups per column
TPP = 8            # tiles per pack (sel8 accumulation group)
NPACK = 31         # packs per core
NTILES = NPACK * TPP          # 248
RPT = G * NT                  # rows per tile (1024)
R = NTILES * RPT              # rows per core (253952)
C = 2048
F32 = mybir.dt.float32
BF16 = mybir.dt.bfloat16
BF = ml_dtypes.bfloat16

_CACHE = {}


def _build_program():
    nc = bacc.Bacc("TRN2", target_bir_lowering=False, debug=False, num_devices=M)

    xin = nc.dram_tensor("xin", [32, NTILES * NT], BF16, kind="ExternalInput")
    yin = nc.dram_tensor("yin", [NPACK * 16, NT], F32, kind="ExternalInput")
    a1 = nc.dram_tensor("a1", [24, 128], BF16, kind="ExternalInput")
    a2 = nc.dram_tensor("a2", [24, 128], BF16, kind="ExternalInput")
    a3 = nc.dram_tensor("a3", [128, 128], BF16, kind="ExternalInput")
    a4 = nc.dram_tensor("a4", [128, TPP * 48], BF16, kind="ExternalInput")
    z = nc.dram_tensor("z", [NPACK * 16, NT], F32, kind="ExternalOutput")

    isq = mybir.AluOpType.is_equal
    mul = mybir.AluOpType.mult
    add = mybir.AluOpType.add

    with TileContext(nc) as tc:
        with (
            tc.tile_pool(name="const", bufs=1) as cpool,
            tc.tile_pool(name="in", bufs=4) as ipool,
            tc.tile_pool(name="mid", bufs=6) as spool,
            tc.tile_pool(name="msk", bufs=TPP + 2) as mpool,
            tc.tile_pool(name="out", bufs=2) as opool,
            tc.tile_pool(name="ps", bufs=2, space="PSUM") as ppool,
            tc.tile_pool(name="psv", bufs=3, space="PSUM") as ppoolv,
            tc.tile_pool(name="pss", bufs=1, space="PSUM") as ppools,
        ):
            a1_t = cpool.tile([24, 128], BF16)
            nc.sync.dma_start(out=a1_t[:], in_=a1[:, :])
            a2_t = cpool.tile([24, 128], BF16)
            nc.sync.dma_start(out=a2_t[:], in_=a2[:, :])
            a3_t = cpool.tile([128, 128], BF16)
            nc.sync.dma_start(out=a3_t[:], in_=a3[:, :])
            a4_t = cpool.tile([128, TPP * 48], BF16)
            nc.sync.dma_start(out=a4_t[:], in_=a4[:, :])

            for pk in range(NPACK):
                xt = ipool.tile([32, TPP * NT], BF16, tag="x")
                nc.sync.dma_start(
                    out=xt[:], in_=xin[:, pk * TPP * NT:(pk + 1) * TPP * NT]
                )
                yt = ipool.tile([16, NT], F32, tag="y")
                nc.sync.dma_start(out=yt[:], in_=yin[16 * pk:16 * (pk + 1), :])

                sel8 = ppools.tile([64, NT], F32, tag="sel")
                msks = []
                for up in range(TPP // 2):
                    u0, u1 = 2 * up, 2 * up + 1
                    xv0 = xt[0:24, u0 * NT:(u0 + 1) * NT]
                    xv1 = xt[0:24, u1 * NT:(u1 + 1) * NT]
                    # ulo in cols [0,NT), uhi in cols [NT,2NT) (adjacent banks);
                    # batch same-stationary matmuls to cut LDWEIGHTS switches
                    uuA = ppool.tile([128, 2 * NT], F32, tag="uu")
                    uuB = ppool.tile([128, 2 * NT], F32, tag="uu")
                    nc.tensor.matmul(
                        out=uuA[:, 0:NT], lhsT=a1_t[:], rhs=xv0, start=True, stop=True
                    )
                    nc.tensor.matmul(
                        out=uuB[:, 0:NT], lhsT=a1_t[:], rhs=xv1, start=True, stop=True
                    )
                    nc.tensor.matmul(
                        out=uuA[:, NT:2 * NT], lhsT=a2_t[:], rhs=xv0,
                        start=True, stop=True
                    )
                    nc.tensor.matmul(
                        out=uuB[:, NT:2 * NT], lhsT=a2_t[:], rhs=xv1,
                        start=True, stop=True
                    )
                    # split drains: lo halves first so the is_equal (and the
                    # V matmul behind it) never waits on the hi-half copy
                    ulobA = spool.tile([128, NT], BF16, tag="ulob")
                    nc.scalar.copy(out=ulobA[:], in_=uuA[:, 0:NT])
                    ulobB = spool.tile([128, NT], BF16, tag="ulob")
                    nc.scalar.copy(out=ulobB[:], in_=uuB[:, 0:NT])
                    uhibA = spool.tile([128, NT], BF16, tag="uhib")
                    nc.scalar.copy(out=uhibA[:], in_=uuA[:, NT:2 * NT])
                    uhibB = spool.tile([128, NT], BF16, tag="uhib")
                    nc.scalar.copy(out=uhibB[:], in_=uuB[:, NT:2 * NT])
                    ohA = spool.tile([128, NT], BF16, tag="oh")
                    nc.vector.tensor_scalar(
                        out=ohA[:], in0=ulobA[:], scalar1=0.0,
                        scalar2=None, op0=isq
                    )
                    ohB = spool.tile([128, NT], BF16, tag="oh")
                    nc.vector.tensor_scalar(
                        out=ohB[:], in0=ulobB[:], scalar1=0.0,
                        scalar2=None, op0=isq
                    )
                    VA = ppoolv.tile([128, NT], F32, tag="V")
                    nc.tensor.matmul(
                        out=VA[:], lhsT=a3_t[:], rhs=ohA[:], start=True, stop=True
                    )
                    VB = ppoolv.tile([128, NT], F32, tag="V")
                    nc.tensor.matmul(
                        out=VB[:], lhsT=a3_t[:], rhs=ohB[:], start=True, stop=True
                    )
                    mskA = mpool.tile([128, NT], BF16, tag="msk")
                    nc.vector.scalar_tensor_tensor(
                        out=mskA[:], in0=uhibA[:], scalar=0.0,
                        in1=VA[:], op0=isq, op1=mul,
                    )
                    mskB = mpool.tile([128, NT], BF16, tag="msk")
                    nc.vector.scalar_tensor_tensor(
                        out=mskB[:], in0=uhibB[:], scalar=0.0,
                        in1=VB[:], op0=isq, op1=mul,
                    )
                    msks.append(mskA)
                    msks.append(mskB)

                for u in range(TPP):
                    nc.tensor.matmul(
                        out=sel8[0:48, :], lhsT=a4_t[:, 48 * u:48 * (u + 1)],
                        rhs=msks[u][:], start=(u == 0), stop=(u == TPP - 1),
                    )

                # z = y*w + b ; w on sel8 lanes [0:16), b on [32:48)
                zt = opool.tile([16, NT], F32, tag="z")
                nc.vector.tensor_tensor(
                    out=zt[:], in0=yt[:], in1=sel8[0:16, :], op=mul
                )
                nc.vector.tensor_tensor(
                    out=zt[:], in0=zt[:], in1=sel8[32:48, :], op=add
                )
                nc.sync.dma_start(out=z[16 * pk:16 * (pk + 1), :], in_=zt[:])

    nc.compile()
    return nc


def _get_program():
    if "nc" not in _CACHE:
        _CACHE["nc"] = _build_program()
    return _CACHE["nc"]


def _make_consts(w, b):
    f32 = np.float32
    wb_ = np.stack([np.asarray(w, f32), np.asarray(b, f32)], 1).astype(BF)  # [2048, 2]
    a1 = np.zeros((24, 128), BF)
    a2 = np.zeros((24, 128), BF)
    for g in range(G):
        for s in range(6):            # x4..x9 -> lo6, coef 32..1
            a1[12 * g + 5 + s, 64 * g:64 * (g + 1)] = BF(2.0 ** (5 - s))
        a1[12 * g + 11, 64 * g:64 * (g + 1)] = -np.arange(64, dtype=f32).astype(BF)
        for s in range(5):            # t,x0..x3 -> hi5, coef 16..1
            a2[12 * g + s, 64 * g:64 * (g + 1)] = BF(2.0 ** (4 - s))
        hvals = np.repeat(np.arange(32, dtype=f32), 2)
        a2[12 * g + 11, 64 * g:64 * (g + 1)] = (-hvals).astype(BF)
    a3 = np.zeros((128, 128), BF)
    for g in range(G):
        for h in range(32):
            for e in range(2):
                a3[64 * g:64 * g + 64, 64 * g + 2 * h + e] = wb_[h * 64:(h + 1) * 64, e]
    # sel8 slot for tile u, group g: w at partition 2u+g, b at 32+2u+g
    a4 = np.zeros((128, TPP * 48), BF)
    for u in range(TPP):
        for g in range(G):
            for e in range(2):
                for h in range(32):
                    a4[64 * g + 2 * h + e, 48 * u + 32 * e + 2 * u + g] = 1.0
    return a1, a2, a3, a4


def kernel(x, t, y, w, b, trace=False):
    N = x.shape[0]
    Npad = M * R
    npad = Npad - N
    assert npad >= 0
    f32 = np.float32

    # features [12, Npad]: t, x0..x9, ones (bf16; all exact)
    F = np.zeros((12, Npad), BF)
    F[0, :N] = np.asarray(t, f32).reshape(-1).astype(BF)
    F[1:11, :N] = np.asarray(x, f32).T.astype(BF)
    F[11, :N] = BF(1.0)

    xin = np.zeros((M, 32, NTILES * NT), BF)
    yp = np.concatenate([np.asarray(y, f32).reshape(-1), np.zeros(npad, f32)])
    yin = np.empty((M, NPACK * 16, NT), f32)
    for m in range(M):
        Fm = F[:, m * R:(m + 1) * R].reshape(12, NTILES, G, NT)
        xin[m, 0:24] = Fm.transpose(2, 0, 1, 3).reshape(24, NTILES * NT)
        yin[m] = yp[m * R:(m + 1) * R].reshape(NPACK * 16, NT)

    a1, a2, a3, a4 = _make_consts(w, b)

    nc = _get_program()
    in_maps = [
        {"xin": xin[i], "yin": yin[i], "a1": a1, "a2": a2, "a3": a3, "a4": a4}
        for i in range(M)
    ]
    res = run_bass_kernel_spmd(nc, in_maps, core_ids=list(range(M)), trace=trace)
    zfull = np.concatenate(
        [res.results[i]["z"].reshape(-1) for i in range(M)]
    )[:N]
    out = zfull.reshape(N, 1).astype(np.float32)
    if trace:
        return out, res
    return out


# revision 16
# speedup vs baseline: 1.0578x; 1.0578x over previous
"""Trainium2 Bass kernel: per-combination linear encoder via PE one-hot
matmuls (no GPSIMD gather).

z = y * w[idx] + b[idx],  idx = t*1024 + x @ [512..1]  (11 bits, 2048 combos)

Split idx = hi5*64 + lo6 (hi5 = t,x0..x3; lo6 = x4..x9). Per tile of
1024 rows (2 groups g of 512 columns, rows on the FREE axis):

  S1a  PE   u_lo[64g+l, c]   = lo6(r) - l       (block-diag affine MM ->
  S1b  PE   u_hi[64g+2h+e,c] = hi5(r) - h        one [128,2*NT] psum pair,
                                                 e-duplicated for w/b lanes)
  cp   ACT  uu -> bf16 SBUF (single merged psum drain, exact: |u| < 64)
  S2   DVE  oh = (u_lo == 0)          bf16, 4x mode
  S3   PE   V[64g+2h+e, c] = sum_l T[l,2h+e]*oh    (T = w/b tables, bf16)
  S4   DVE  msk = (u_hi == 0) * V     (fused scalar_tensor_tensor)
  sel  PE   8 tiles accumulate into sel8[48,512]: tile u writes w to
            slot 2u+g, b to slot 32+2u+g via slot-shifted one-hot
            stationaries (M=48, N=512, one accumulation group per pack)
  FMA  DVE  z[16,512] = y16 * sel8[0:16] + sel8[32:48]; DMA out.
Matmuls are pair-batched (same stationary back to back) to cut
LDWEIGHTS switches; ~516 us on 8 cores (PE-paced).

All row/column packing is host-side layout only; the device does all
arithmetic. w/b are bf16 (0.2% rounding, tolerance is 2e-2).
"""

import numpy as np
import ml_dtypes

import concourse.bacc as bacc
import concourse.mybir as mybir
from concourse.tile import TileContext
from concourse.bass_utils import run_bass_kernel_spmd

M = 8              # NeuronCores
NT = 512           # columns per tile (one PSUM bank)
G = 2              # row-groups per column
TPP = 8            # tiles per pack (sel8 accumulation group)
NPACK = 31         # packs per core
NTILES = NPACK * TPP          # 248
RPT = G * NT                  # rows per tile (1024)
R = NTILES * RPT              # rows per core (253952)
C = 2048
F32 = mybir.dt.float32
BF16 = mybir.dt.bfloat16
BF = ml_dtypes.bfloat16

_CACHE = {}


def _build_program():
    nc = bacc.Bacc("TRN2", target_bir_lowering=False, debug=False, num_devices=M)

    xin = nc.dram_tensor("xin", [32, NTILES * NT], BF16, kind="ExternalInput")
    yin = nc.dram_tensor("yin", [NPACK * 16, NT], F32, kind="ExternalInput")
    a1 = nc.dram_tensor("a1", [24, 128], BF16, kind="ExternalInput")
    a2 = nc.dram_tensor("a2", [24, 128], BF16, kind="ExternalInput")
    a3 = nc.dram_tensor("a3", [128, 128], BF16, kind="ExternalInput")
    a4 = nc.dram_tensor("a4", [128, TPP * 48], BF16, kind="ExternalInput")
    z = nc.dram_tensor("z", [NPACK * 16, NT], F32, kind="ExternalOutput")

    isq = mybir.AluOpType.is_equal
    mul = mybir.AluOpType.mult
    add = mybir.AluOpType.add

    with TileContext(nc) as tc:
        with (
            tc.tile_pool(name="const", bufs=1) as cpool,
            tc.tile_pool(name="in", bufs=4) as ipool,
            tc.tile_pool(name="mid", bufs=6) as spool,
            tc.tile_pool(name="msk", bufs=TPP + 2) as mpool,
            tc.tile_pool(name="out", bufs=2) as opool,
            tc.tile_pool(name="ps", bufs=2, space="PSUM") as ppool,
            tc.tile_pool(name="psv", bufs=3, space="PSUM") as ppoolv,
            tc.tile_pool(name="pss", bufs=1, space="PSUM") as ppools,
        ):
            a1_t = cpool.tile([24, 128], BF16)
            nc.sync.dma_start(out=a1_t[:], in_=a1[:, :])
            a2_t = cpool.tile([24, 128], BF16)
            nc.sync.dma_start(out=a2_t[:], in_=a2[:, :])
            a3_t = cpool.tile([128, 128], BF16)
            nc.sync.dma_start(out=a3_t[:], in_=a3[:, :])
            a4_t = cpool.tile([128, TPP * 48], BF16)
            nc.sync.dma_start(out=a4_t[:], in_=a4[:, :])

            for pk in range(NPACK):
                xt = ipool.tile([32, TPP * NT], BF16, tag="x")
                nc.sync.dma_start(
                    out=xt[:], in_=xin[:, pk * TPP * NT:(pk + 1) * TPP * NT]
                )
                yt = ipool.tile([16, NT], F32, tag="y")
                nc.sync.dma_start(out=yt[:], in_=yin[16 * pk:16 * (pk + 1), :])

                sel8 = ppools.tile([64, NT], F32, tag="sel")
                msks = []
                for up in range(TPP // 2):
                    u0, u1 = 2 * up, 2 * up + 1
                    xv0 = xt[0:24, u0 * NT:(u0 + 1) * NT]
                    xv1 = xt[0:24, u1 * NT:(u1 + 1) * NT]
                    # ulo in cols [0,NT), uhi in cols [NT,2NT) (adjacent banks);
                    # batch same-stationary matmuls to cut LDWEIGHTS switches
                    uuA = ppool.tile([128, 2 * NT], F32, tag="uu")
                    uuB = ppool.tile([128, 2 * NT], F32, tag="uu")
                    nc.tensor.matmul(
                        out=uuA[:, 0:NT], lhsT=a1_t[:], rhs=xv0, start=True, stop=True
                    )
                    nc.tensor.matmul(
                        out=uuB[:, 0:NT], lhsT=a1_t[:], rhs=xv1, start=True, stop=True
                    )
                    nc.tensor.matmul(
                        out=uuA[:, NT:2 * NT], lhsT=a2_t[:], rhs=xv0,
                        start=True, stop=True
                    )
                    nc.tensor.matmul(
                        out=uuB[:, NT:2 * NT], lhsT=a2_t[:], rhs=xv1,
                        start=True, stop=True
                    )
                    uubA = spool.tile([128, 2 * NT], BF16, tag="uub")
                    nc.scalar.copy(out=uubA[:], in_=uuA[:])
                    uubB = spool.tile([128, 2 * NT], BF16, tag="uub")
                    nc.scalar.copy(out=uubB[:], in_=uuB[:])
                    ohA = spool.tile([128, NT], BF16, tag="oh")
                    nc.vector.tensor_scalar(
                        out=ohA[:], in0=uubA[:, 0:NT], scalar1=0.0,
                        scalar2=None, op0=isq
                    )
                    ohB = spool.tile([128, NT], BF16, tag="oh")
                    nc.vector.tensor_scalar(
                        out=ohB[:], in0=uubB[:, 0:NT], scalar1=0.0,
                        scalar2=None, op0=isq
                    )
                    VA = ppoolv.tile([128, NT], F32, tag="V")
                    nc.tensor.matmul(
                        out=VA[:], lhsT=a3_t[:], rhs=ohA[:], start=True, stop=True
                    )
                    VB = ppoolv.tile([128, NT], F32, tag="V")
                    nc.tensor.matmul(
                        out=VB[:], lhsT=a3_t[:], rhs=ohB[:], start=True, stop=True
                    )
                    mskA = mpool.tile([128, NT], BF16, tag="msk")
                    nc.vector.scalar_tensor_tensor(
                        out=mskA[:], in0=uubA[:, NT:2 * NT], scalar=0.0,
                        in1=VA[:], op0=isq, op1=mul,
                    )
                    mskB = mpool.tile([128, NT], BF16, tag="msk")
                    nc.vector.scalar_tensor_tensor(
                        out=mskB[:], in0=uubB[:, NT:2 * NT], scalar=0.0,
                        in1=VB[:], op0=isq, op1=mul,
                    )
                    msks.append(mskA)
                    msks.append(mskB)

                for u in range(TPP):
                    nc.tensor.matmul(
                        out=sel8[0:48, :], lhsT=a4_t[:, 48 * u:48 * (u + 1)],
                        rhs=msks[u][:], start=(u == 0), stop=(u == TPP - 1),
                    )

                # z = y*w + b ; w on sel8 lanes [0:16), b on [32:48)
                zt = opool.tile([16, NT], F32, tag="z")
                nc.vector.tensor_tensor(
                    out=zt[:], in0=yt[:], in1=sel8[0:16, :], op=mul
                )
                nc.vector.tensor_tensor(
                    out=zt[:], in0=zt[:], in1=sel8[32:48, :], op=add
                )
                nc.sync.dma_start(out=z[16 * pk:16 * (pk + 1), :], in_=zt[:])

    nc.compile()
    return nc


def _get_program():
    if "nc" not in _CACHE:
        _CACHE["nc"] = _build_program()
    return _CACHE["nc"]


def _make_consts(w, b):
    f32 = np.float32
    wb_ = np.stack([np.asarray(w, f32), np.asarray(b, f32)], 1).astype(BF)  # [2048, 2]
    a1 = np.zeros((24, 128), BF)
    a2 = np.zeros((24, 128), BF)
    for g in range(G):
        for s in range(6):            # x4..x9 -> lo6, coef 32..1
            a1[12 * g + 5 + s, 64 * g:64 * (g + 1)] = BF(2.0 ** (5 - s))
        a1[12 * g + 11, 64 * g:64 * (g + 1)] = -np.arange(64, dtype=f32).astype(BF)
        for s in range(5):            # t,x0..x3 -> hi5, coef 16..1
            a2[12 * g + s, 64 * g:64 * (g + 1)] = BF(2.0 ** (4 - s))
        hvals = np.repeat(np.arange(32, dtype=f32), 2)
        a2[12 * g + 11, 64 * g:64 * (g + 1)] = (-hvals).astype(BF)
    a3 = np.zeros((128, 128), BF)
    for g in range(G):
        for h in range(32):
            for e in range(2):
                a3[64 * g:64 * g + 64, 64 * g + 2 * h + e] = wb_[h * 64:(h + 1) * 64, e]
    # sel8 slot for tile u, group g: w at partition 2u+g, b at 32+2u+g
    a4 = np.zeros((128, TPP * 48), BF)
    for u in range(TPP):
        for g in range(G):
            for e in range(2):
                for h in range(32):
                    a4[64 * g + 2 * h + e, 48 * u + 32 * e + 2 * u + g] = 1.0
    return a1, a2, a3, a4


def kernel(x, t, y, w, b, trace=False):
    N = x.shape[0]
    Npad = M * R
    npad = Npad - N
    assert npad >= 0
    f32 = np.float32

    # features [12, Npad]: t, x0..x9, ones (bf16; all exact)
    F = np.zeros((12, Npad), BF)
    F[0, :N] = np.asarray(t, f32).reshape(-1).astype(BF)
    F[1:11, :N] = np.asarray(x, f32).T.astype(BF)
    F[11, :N] = BF(1.0)

    xin = np.zeros((M, 32, NTILES * NT), BF)
    yp = np.concatenate([np.asarray(y, f32).reshape(-1), np.zeros(npad, f32)])
    yin = np.empty((M, NPACK * 16, NT), f32)
    for m in range(M):
        Fm = F[:, m * R:(m + 1) * R].reshape(12, NTILES, G, NT)
        xin[m, 0:24] = Fm.transpose(2, 0, 1, 3).reshape(24, NTILES * NT)
        yin[m] = yp[m * R:(m + 1) * R].reshape(NPACK * 16, NT)

    a1, a2, a3, a4 = _make_consts(w, b)

    nc = _get_program()
    in_maps = [
        {"xin": xin[i], "yin": yin[i], "a1": a1, "a2": a2, "a3": a3, "a4": a4}
        for i in range(M)
    ]
    res = run_bass_kernel_spmd(nc, in_maps, core_ids=list(range(M)), trace=trace)
    zfull = np.concatenate(
        [res.results[i]["z"].reshape(-1) for i in range(M)]
    )[:N]
    out = zfull.reshape(N, 1).astype(np.float32)
    if trace:
        return out, res
    return out
